# revision 28
# baseline (speedup 1.0000x reference)
"""v10: one fp16 matmul per 4 samples. Host packs samples at 32-col strides so
each 128-col weight load (FWL-eligible) covers 4 samples on aligned strips;
each pack writes a bank-aligned 128-col PSUM block (an MM output must not
cross a 2KB PSUM bank boundary); diagonal 27x27 blocks extracted on DVE+Act;
strip-major fp16 gram output."""

import os
import sys

import numpy as np

for _p in (
    "/root/.axon_site",
    "/root/.axon_site/_ro/trn_rl_repo",
    "/opt/trn_rl_repo",
):
    if os.path.isdir(_p) and _p not in sys.path:
        sys.path.append(_p)

import concourse.bacc as bacc
import concourse.mybir as mybir
import concourse.tile as tile

NF = 27
NP32 = 32  # per-sample column pitch in the padded layout
D = 128
B = 32768
NCORES = 8
S = B // NCORES

F16 = mybir.dt.float16
F32 = mybir.dt.float32

TOFF = np.concatenate([[0], np.cumsum(NF - np.arange(NF))]).astype(np.int64)
NPAIRS = int(TOFF[NF])
DOUT = D + NPAIRS

PACK = 4  # samples per matmul (4 x 32 = 128 weight cols)
PPT = 8  # packs per PSUM tile (8 x 4 x 27 = 864 fp32 = 2 banks)


def build_nc(s_per_core=S):
    # progressive chunk sizes: small first chunks so matmuls start while the
    # bulk of the input is still streaming; small last chunk to shrink the tail
    chunk_sizes = [64, 64, 128, 256] + [512] * 6 + [256, 128, 64, 64]
    assert sum(chunk_sizes) == s_per_core

    nc = bacc.Bacc("TRN2", target_bir_lowering=False, debug=False)
    xt = nc.dram_tensor("xt", [D, s_per_core * NP32], F16, kind="ExternalInput")
    gram = nc.dram_tensor(
        "gram", [PACK, NF, s_per_core // PACK, NF], F16, kind="ExternalOutput"
    )

    with tile.TileContext(nc) as tc:
        with (
            tc.tile_pool(name="xin", bufs=5) as xin_pool,
            tc.tile_pool(name="gbuf", bufs=3) as gbuf_pool,
            tc.tile_pool(name="ps", bufs=4, space="PSUM") as ps_pool,
        ):
            dma_engines = [nc.gpsimd, nc.sync, nc.scalar]
            rr = [0]
            cc = [0]
            s_base = 0
            for c_sz in chunk_sizes:
                packs_per_chunk = c_sz // PACK
                tiles_per_chunk = packs_per_chunk // PPT
                pack_base = s_base // PACK
                gbuf = gbuf_pool.tile([128, packs_per_chunk * NF], F16, tag="gbuf")
                xin = xin_pool.tile([D, c_sz * NP32], F16, tag="xin")
                eng = dma_engines[rr[0] % 3]
                rr[0] += 1
                eng.dma_start(
                    out=xin[:],
                    in_=xt[:, s_base * NP32 : (s_base + c_sz) * NP32],
                )
                for t in range(tiles_per_chunk):
                    # one bank-aligned 128-col PSUM block per pack (no MM
                    # output may cross a 2KB PSUM bank boundary)
                    ps = ps_pool.tile([128, PPT * 128], F32)
                    for q in range(PPT):
                        loc = (t * PPT + q) * PACK * NP32
                        wsl = xin[:, loc : loc + 128]
                        nc.tensor.matmul(
                            ps[:, q * 128 : (q + 1) * 128],
                            wsl,
                            wsl,
                            start=True,
                            stop=True,
                        )
                    # extract diagonal 27x27 blocks: strip 32l holds sample 4p+l
                    for l in range(PACK):
                        src = ps[32 * l : 32 * l + NF, :].rearrange(
                            "p (q c) -> p q c", q=PPT
                        )[:, :, 32 * l : 32 * l + NF]
                        dst = gbuf[
                            32 * l : 32 * l + NF,
                            t * PPT * NF : (t + 1) * PPT * NF,
                        ].rearrange("p (q m) -> p q m", q=PPT)
                        if cc[0] % 2 == 0:
                            nc.vector.tensor_copy(dst, src)
                        else:
                            nc.scalar.copy(dst, src)
                        cc[0] += 1
                for l in range(PACK):
                    eng2 = dma_engines[rr[0] % 3]
                    rr[0] += 1
                    eng2.dma_start(
                        out=gram[
                            l,
                            :,
                            pack_base : pack_base + packs_per_chunk,
                            :,
                        ],
                        in_=gbuf[32 * l : 32 * l + NF, : packs_per_chunk * NF],
                    )
                s_base += c_sz
    nc.finalize()
    return nc


def host_pack_inputs(dense_features, sparse_features):
    bsz = dense_features.shape[0]
    xt = np.zeros((D, bsz, NP32), dtype=np.float16)
    xt[:, :, 0] = np.asarray(dense_features, dtype=np.float32).T
    xt[:, :, 1:NF] = np.asarray(sparse_features, dtype=np.float32).transpose(2, 0, 1)
    return xt


def host_core_input(xt, c, s_per_core=S):
    return np.ascontiguousarray(
        xt[:, c * s_per_core : (c + 1) * s_per_core, :]
    ).reshape(D, s_per_core * NP32)


def host_unpack_output(dense_features, gram_t):
    """gram_t: [PACK, NF, B//PACK, NF]; sample 4k+l lives at gram_t[l, :, k, :]."""
    bsz = dense_features.shape[0]
    out = np.empty((bsz, DOUT), dtype=np.float32)
    out[:, :D] = dense_features
    gram_t = gram_t.astype(np.float32)
    for l in range(PACK):
        for n in range(NF):
            lo = D + int(TOFF[n])
            out[l::PACK, lo : lo + NF - n] = gram_t[l, n, :, n:]
    return out


_NC_CACHE = {}


def _get_nc():
    key = (S,)
    if key not in _NC_CACHE:
        _NC_CACHE[key] = build_nc(S)
    return _NC_CACHE[key]


def kernel(dense_features, sparse_features):
    from concourse.bass_utils import run_bass_kernel_spmd

    dense_features = np.asarray(dense_features, dtype=np.float32)
    sparse_features = np.asarray(sparse_features, dtype=np.float32)
    xt = host_pack_inputs(dense_features, sparse_features)
    in_maps = [{"xt": host_core_input(xt, c)} for c in range(NCORES)]
    nc = _get_nc()
    res = run_bass_kernel_spmd(nc, in_maps, core_ids=list(range(NCORES)))
    gram_t = np.concatenate([r["gram"] for r in res.results], axis=2)
    return host_unpack_output(dense_features, gram_t)


# revision 30
# speedup vs baseline: 1.0177x; 1.0177x over previous
"""v10: one fp16 matmul per 4 samples. Host packs samples at 32-col strides so
each 128-col weight load (FWL-eligible) covers 4 samples on aligned strips;
each pack writes a bank-aligned 128-col PSUM block (an MM output must not
cross a 2KB PSUM bank boundary); diagonal 27x27 blocks extracted on DVE+Act;
strip-major fp16 gram output."""

import os
import sys

import numpy as np

for _p in (
    "/root/.axon_site",
    "/root/.axon_site/_ro/trn_rl_repo",
    "/opt/trn_rl_repo",
):
    if os.path.isdir(_p) and _p not in sys.path:
        sys.path.append(_p)

import concourse.bacc as bacc
import concourse.mybir as mybir
import concourse.tile as tile

NF = 27
NP32 = 32  # per-sample column pitch in the padded layout
D = 128
B = 32768
NCORES = 8
S = B // NCORES

F16 = mybir.dt.float16
F32 = mybir.dt.float32

TOFF = np.concatenate([[0], np.cumsum(NF - np.arange(NF))]).astype(np.int64)
NPAIRS = int(TOFF[NF])
DOUT = D + NPAIRS

PACK = 4  # samples per matmul (4 x 32 = 128 weight cols)
PPT = 8  # packs per PSUM tile (8 x 4 x 27 = 864 fp32 = 2 banks)


def build_nc(s_per_core=S):
    # progressive chunk sizes: small first chunks so matmuls start while the
    # bulk of the input is still streaming; small last chunk to shrink the tail
    chunk_sizes = [128, 128, 256] + [512] * 6 + [256, 128, 128]
    assert sum(chunk_sizes) == s_per_core

    nc = bacc.Bacc("TRN2", target_bir_lowering=False, debug=False)
    xt = nc.dram_tensor("xt", [D, s_per_core * NP32], F16, kind="ExternalInput")
    gram = nc.dram_tensor(
        "gram", [PACK, NF, s_per_core // PACK, NF], F16, kind="ExternalOutput"
    )

    with tile.TileContext(nc) as tc:
        with (
            tc.tile_pool(name="xin", bufs=4) as xin_pool,
            tc.tile_pool(name="gbuf", bufs=3) as gbuf_pool,
            tc.tile_pool(name="ps", bufs=4, space="PSUM") as ps_pool,
        ):
            dma_engines = [nc.gpsimd, nc.sync, nc.scalar]
            rr = [0]
            cc = [0]
            s_base = 0
            for c_sz in chunk_sizes:
                packs_per_chunk = c_sz // PACK
                tiles_per_chunk = packs_per_chunk // PPT
                pack_base = s_base // PACK
                gbuf = gbuf_pool.tile([128, packs_per_chunk * NF], F16, tag="gbuf")
                xin = xin_pool.tile([D, c_sz * NP32], F16, tag="xin")
                eng = dma_engines[rr[0] % 3]
                rr[0] += 1
                eng.dma_start(
                    out=xin[:],
                    in_=xt[:, s_base * NP32 : (s_base + c_sz) * NP32],
                )
                for t in range(tiles_per_chunk):
                    # one bank-aligned 128-col PSUM block per pack (no MM
                    # output may cross a 2KB PSUM bank boundary)
                    ps = ps_pool.tile([128, PPT * 128], F32)
                    for q in range(PPT):
                        loc = (t * PPT + q) * PACK * NP32
                        wsl = xin[:, loc : loc + 128]
                        nc.tensor.matmul(
                            ps[:, q * 128 : (q + 1) * 128],
                            wsl,
                            wsl,
                            start=True,
                            stop=True,
                        )
                    # extract diagonal 27x27 blocks: strip 32l holds sample 4p+l
                    for l in range(PACK):
                        src = ps[32 * l : 32 * l + NF, :].rearrange(
                            "p (q c) -> p q c", q=PPT
                        )[:, :, 32 * l : 32 * l + NF]
                        dst = gbuf[
                            32 * l : 32 * l + NF,
                            t * PPT * NF : (t + 1) * PPT * NF,
                        ].rearrange("p (q m) -> p q m", q=PPT)
                        if cc[0] % 9 < 5:
                            nc.vector.tensor_copy(dst, src)
                        else:
                            nc.scalar.copy(dst, src)
                        cc[0] += 1
                out_engines = [nc.gpsimd, nc.sync]
                for l in range(PACK):
                    eng2 = out_engines[rr[0] % 2]
                    rr[0] += 1
                    eng2.dma_start(
                        out=gram[
                            l,
                            :,
                            pack_base : pack_base + packs_per_chunk,
                            :,
                        ],
                        in_=gbuf[32 * l : 32 * l + NF, : packs_per_chunk * NF],
                    )
                s_base += c_sz
    nc.finalize()
    return nc


def host_pack_inputs(dense_features, sparse_features):
    bsz = dense_features.shape[0]
    xt = np.zeros((D, bsz, NP32), dtype=np.float16)
    xt[:, :, 0] = np.asarray(dense_features, dtype=np.float32).T
    xt[:, :, 1:NF] = np.asarray(sparse_features, dtype=np.float32).transpose(2, 0, 1)
    return xt


def host_core_input(xt, c, s_per_core=S):
    return np.ascontiguousarray(
        xt[:, c * s_per_core : (c + 1) * s_per_core, :]
    ).reshape(D, s_per_core * NP32)


def host_unpack_output(dense_features, gram_t):
    """gram_t: [PACK, NF, B//PACK, NF]; sample 4k+l lives at gram_t[l, :, k, :]."""
    bsz = dense_features.shape[0]
    out = np.empty((bsz, DOUT), dtype=np.float32)
    out[:, :D] = dense_features
    gram_t = gram_t.astype(np.float32)
    for l in range(PACK):
        for n in range(NF):
            lo = D + int(TOFF[n])
            out[l::PACK, lo : lo + NF - n] = gram_t[l, n, :, n:]
    return out


_NC_CACHE = {}


def _get_nc():
    key = (S,)
    if key not in _NC_CACHE:
        _NC_CACHE[key] = build_nc(S)
    return _NC_CACHE[key]


def kernel(dense_features, sparse_features):
    from concourse.bass_utils import run_bass_kernel_spmd

    dense_features = np.asarray(dense_features, dtype=np.float32)
    sparse_features = np.asarray(sparse_features, dtype=np.float32)
    xt = host_pack_inputs(dense_features, sparse_features)
    in_maps = [{"xt": host_core_input(xt, c)} for c in range(NCORES)]
    nc = _get_nc()
    res = run_bass_kernel_spmd(nc, in_maps, core_ids=list(range(NCORES)))
    gram_t = np.concatenate([r["gram"] for r in res.results], axis=2)
    return host_unpack_output(dense_features, gram_t)
